# revision 18
# baseline (speedup 1.0000x reference)
"""Trainium2 Bass kernel: single-layer causal attention block (q/k/v/o + RoPE).

Sharding: 8 cores = 2 batches x 4 head-groups (4 heads each).
Per core (SPMD, differs only in input data), all matmul operands bf16:
  - q/k projections emit a merged per-head [even(32); odd(32)] row layout so
    each score tile is ONE 64-contraction matmul (vs 2x32 split).
  - RoPE: partner rows fetched via a PE permutation matmul (row r <- r^32),
    then dst = A*CS + rot*SN where the rot*SN mul reads the permute psum
    directly (no staging copy).
  - scores^T = K-stationary matmul -> exp (bf16) on ACT -> multiplicative
    causal mask on diagonal tiles -> PV with a ones-column rowsum.
  - softmax normalization batched: rowsums -> one reciprocal_approx_fast,
    per-(head,chunk) broadcast via gpsimd partition_broadcast.
  - o_proj partial [2048, 1024] emitted in bf16.

FUSED SINGLE-PASS SCHEDULE (v2): q-chunks are processed ASCENDING (0..3) and
the projections for chunk c+1 (q/k/rope/v) are emitted after chunk c's
attention.  The Tile scheduler (priority = emission order, ready-instruction
skip-ahead) then interleaves projection matmuls into the PE whenever the
attention stream stalls on the ACT exp pipeline, keeping the PE dense so the
HAM clock gate stays at 8/8 (the two-phase version spent ~100us at half
clock waiting on exp at k-tile granularity).  o_proj(c) is deferred into
chunk c+1's attention as additional PE filler; o_proj(2) lands mid-chunk-3
where the exp load peaks.

Host: sums the 4 per-head-group partials per batch (row-sharded o_proj
unshard) in f32 and stacks the 2 batches.
"""

import os
import sys

import numpy as np

sys.path.insert(0, "/opt/trn_rl_repo")

import concourse.bass as bass  # noqa: E402
import concourse.tile as tile  # noqa: E402
from concourse import bacc, mybir  # noqa: E402
from concourse import bass_utils  # noqa: E402

B, S, D, H, DK = 2, 2048, 1024, 16, 64
NCORES = 8
HPC = H // 4  # 4 heads per core
CW = HPC * DK  # 256 head-dim columns per core
VW = DK + 1  # 65: v width per head incl ones column
ND = D // 128  # 8 contraction chunks
NS = S // 128  # 16 s-tiles
NSC = S // 512  # 4 s-chunks
ROPE_THETA = 10000.0

F32 = mybir.dt.float32
BF16 = mybir.dt.bfloat16
EXP = mybir.ActivationFunctionType.Exp


def _build_kernel(tc, nc, xt, wq, wk, wv, wo, cs, sn, mk, pm, out):
    from contextlib import ExitStack
    _stack = ExitStack()
    constp = _stack.enter_context(tc.tile_pool(name="const", bufs=1))
    pers = _stack.enter_context(tc.tile_pool(name="persist", bufs=1))
    xtp = _stack.enter_context(tc.tile_pool(name="xt", bufs=1))

    wq_sb = constp.tile([128, ND * CW], BF16)
    wk_sb = constp.tile([128, ND * CW], BF16)
    wv_sb = constp.tile([128, ND * CW], BF16)
    wo_sb = constp.tile([128, 2 * D], BF16)
    cs_sb = constp.tile([128, S], BF16)
    sn_sb = constp.tile([128, S], BF16)
    mk_sb = constp.tile([128, 1024], BF16)
    pm_sb = constp.tile([128, 128], BF16)
    xts = xtp.tile([128, ND * S], BF16)
    # DMA order tuned for the ascending fused schedule: everything chunk 0
    # needs (wq/wk, x s-chunk 0, rope tables, wv, mask) lands first; later x
    # s-chunks and wo stream in behind.  Two hwdge queues (sync + scalar).
    xts3 = xts[:].rearrange("p (d s) -> p d s", s=S)
    wq3 = wq[:].rearrange("(d p) c -> p d c", p=128)
    wk3 = wk[:].rearrange("(d p) c -> p d c", p=128)
    wq_sb3 = wq_sb[:].rearrange("p (d c) -> p d c", c=CW)
    wk_sb3 = wk_sb[:].rearrange("p (d c) -> p d c", c=CW)
    xt3 = xt[:].rearrange("(d p) s -> p d s", p=128)
    # Early critical DMAs.  The scalar queue gets ONLY 5 issues up front: a
    # 6th+ would hit the HWDGE ring limit and BLOCK the ACT engine's stream
    # (observed: araw/exp instructions stuck ~10us behind queue-full DMA
    # issues).  The remaining scalar-queue transfers are emitted later, at
    # points in the ACT stream where their ring slots have long freed.
    nc.sync.dma_start(wq_sb3[:, 0:4], wq3[:, 0:4])
    nc.scalar.dma_start(wq_sb3[:, 4:8], wq3[:, 4:8])
    nc.sync.dma_start(xts3[:, 0:4, 0:512], xt3[:, 0:4, 0:512])
    nc.scalar.dma_start(xts3[:, 4:8, 0:512], xt3[:, 4:8, 0:512])
    nc.sync.dma_start(wk_sb3[:, 0:4], wk3[:, 0:4])
    nc.scalar.dma_start(wk_sb3[:, 4:8], wk3[:, 4:8])
    # rope tables sliced: chunk 0's columns land ~4us before the full table
    # could, shaving the prologue critical path (rope gates the first scores)
    nc.sync.dma_start(cs_sb[:, 0:512], cs[:, 0:512])
    nc.scalar.dma_start(sn_sb[:, 0:512], sn[:, 0:512])
    nc.sync.dma_start(pm_sb[:], pm[:])
    nc.sync.dma_start(mk_sb[:], mk[:])
    nc.scalar.dma_start(wv_sb[:].rearrange("p (d c) -> p d c", c=CW),
                        wv[:].rearrange("(d p) c -> p d c", p=128))
    nc.sync.dma_start(xts3[:, 0:4, 512:1024], xt3[:, 0:4, 512:1024])
    nc.sync.dma_start(cs_sb[:, 512:S], cs[:, 512:S])
    nc.sync.dma_start(wo_sb[:].rearrange("p (b c) -> p b c", c=D),
                      wo[:].rearrange("(b p) c -> p b c", p=128))
    nc.sync.dma_start(xts3[:, 0:4, 1024:1536], xt3[:, 0:4, 1024:1536])
    nc.sync.dma_start(xts3[:, 0:4, 1536:S], xt3[:, 0:4, 1536:S])

    late_dma = [
        lambda: nc.scalar.dma_start(xts3[:, 4:8, 512:1024],
                                    xt3[:, 4:8, 512:1024]),
        lambda: nc.scalar.dma_start(sn_sb[:, 512:S], sn[:, 512:S]),
        lambda: nc.scalar.dma_start(xts3[:, 4:8, 1024:1536],
                                    xt3[:, 4:8, 1024:1536]),
        lambda: nc.scalar.dma_start(xts3[:, 4:8, 1536:S],
                                    xt3[:, 4:8, 1536:S]),
    ]

    # q/k: one [64, S] tile per head, rows = [even(32); odd(32)], so every
    # score matmul runs at tile_position (0,0) (no PE retile between score
    # and PV matmuls).
    qh = [pers.tile([64, S], BF16, name=f"qh{_h}") for _h in range(HPC)]
    kh = [pers.tile([64, S], BF16, name=f"kh{_h}") for _h in range(HPC)]
    v_sb = pers.tile([128, NS * HPC * VW], BF16)
    ctx_sb = pers.tile([128, 2 * S], BF16)
    # rowsum rows live at partition h*32 (engine partition starts must be
    # multiples of 32); unused rows memset to 1.0 so reciprocal stays finite
    sums_sb = pers.tile([128, S], F32)
    rcp_sb = pers.tile([128, S], F32)

    v3 = v_sb[:].rearrange("p (t c) -> p t c", c=VW)
    nc.vector.memset(v3[:, :, DK:DK + 1], 1.0)
    nc.vector.memset(sums_sb[:], 1.0)

    blocks = [(0, 1, wq_sb, 0, True), (2, 3, wq_sb, 1, True),
              (0, 1, wk_sb, 0, False), (2, 3, wk_sb, 1, False)]

    gemm = _stack.enter_context(
        tc.tile_pool(name="gemm", bufs=2, space="PSUM"))
    sps = _stack.enter_context(tc.tile_pool(name="sps", bufs=2, space="PSUM"))
    cps = _stack.enter_context(tc.tile_pool(name="cps", bufs=1, space="PSUM"))
    rsb = _stack.enter_context(tc.tile_pool(name="rsb", bufs=3))
    exp_pool = _stack.enter_context(tc.tile_pool(name="expool", bufs=8))
    nsb = _stack.enter_context(tc.tile_pool(name="nsb", bufs=4))
    obp = _stack.enter_context(tc.tile_pool(name="obuf", bufs=2))

    def proj_qk_block(sc, bi):
        # one projection chain (q or k, 2 heads) + its RoPE tail
        csl = slice(sc * 512, (sc + 1) * 512)
        i0, i1, wsb, blk, is_q = blocks[bi]
        dst0 = qh[i0] if is_q else kh[i0]
        dst1 = qh[i1] if is_q else kh[i1]
        pst = gemm.tile([128, 512], F32, tag="g")
        for d in range(ND):
            nc.tensor.matmul(
                pst[:],
                wsb[:, d * CW + blk * 128: d * CW + blk * 128 + 128],
                xts[:, d * S + sc * 512: d * S + (sc + 1) * 512],
                start=(d == 0), stop=(d == ND - 1),
                skip_group_check=True)
        araw = rsb.tile([128, 512], BF16, tag="araw")
        nc.scalar.copy(araw[:], pst[:])
        prot = gemm.tile([128, 512], F32, tag="g")
        nc.tensor.matmul(prot[:], pm_sb[:], araw[:], start=True,
                         stop=True, skip_group_check=True)
        t1 = rsb.tile([128, 512], BF16, tag="t1")
        t2 = rsb.tile([128, 512], BF16, tag="t2")
        nc.vector.tensor_mul(t1[:], araw[:], cs_sb[:, csl])
        # rot*sn straight from the permute psum (f32 src, no staging)
        nc.vector.tensor_mul(t2[:], prot[:], sn_sb[:, csl])
        nc.vector.tensor_add(dst0[:, csl], t1[0:64, :], t2[0:64, :])
        nc.vector.tensor_add(dst1[:, csl], t1[64:128, :], t2[64:128, :])

    def proj_v_tile(sm):
        pv = gemm.tile([128, CW], F32, tag="g")
        for d in range(ND):
            nc.tensor.matmul(
                pv[:],
                xts[:, d * S + sm * 128: d * S + sm * 128 + 128],
                wv_sb[:, d * CW:(d + 1) * CW],
                start=(d == 0), stop=(d == ND - 1),
                skip_group_check=True)
        base = sm * HPC * VW
        dst3 = v_sb[:, base:base + HPC * VW].rearrange(
            "p (h c) -> p h c", c=VW)
        nc.vector.tensor_copy(dst3[:, :, 0:DK],
                              pv[:].rearrange("p (h c) -> p h c", c=DK))

    def o_proj_sm(sm):
        pos = []
        for do_ in range(2):
            po = gemm.tile([128, 512], F32, tag="g")
            for cb in range(2):
                nc.tensor.matmul(
                    po[:],
                    ctx_sb[:, cb * S + sm * 128: cb * S + sm * 128 + 128],
                    wo_sb[:, cb * D + do_ * 512: cb * D + (do_ + 1) * 512],
                    start=(cb == 0), stop=(cb == 1),
                    skip_group_check=True)
            pos.append(po)
        ot = obp.tile([128, 1024], BF16)
        nc.vector.tensor_copy(ot[:, 0:512], pos[0][:])
        nc.vector.tensor_copy(ot[:, 512:1024], pos[1][:])
        nc.sync.dma_start(out[sm * 128:(sm + 1) * 128, :], ot[:])

    def qk_units(sc):
        # q01/k01 first so the next chunk's hp0 attention unblocks before
        # hp1's projections even start
        return [lambda sc=sc: proj_qk_block(sc, 0),
                lambda sc=sc: proj_qk_block(sc, 2),
                lambda sc=sc: proj_qk_block(sc, 1),
                lambda sc=sc: proj_qk_block(sc, 3)]

    def v_units(sc):
        return [lambda sm=sm: proj_v_tile(sm)
                for sm in range(4 * sc, 4 * sc + 4)]

    def o_units(sms):
        return [lambda sm=sm: o_proj_sm(sm) for sm in sms]

    def attention_hp(c, hp, fillers):
        # Heads (2hp, 2hp+1) share one [128,1024] psum pair-tile per k-tile:
        # two score matmuls, ONE exp over the pair, two PV accumulations.
        nsk = 4 * (c + 1)
        h0, h1 = 2 * hp, 2 * hp + 1
        pctx0 = cps.tile([VW, 512], F32, tag="pctx0")
        pctx1 = cps.tile([VW, 512], F32, tag="pctx1")
        exps = []

        def pv_pair(ta, tb):
            # two consecutive PV accumulations per pctx bank back-to-back:
            # same-bank chains issue fastest on the PE
            for s, (hh, pc) in enumerate(((h0, pctx0), (h1, pctx1))):
                for t in (ta, tb):
                    qo = max(0, t - 4 * c) * 128
                    vbase = t * HPC * VW + hh * VW
                    nc.tensor.matmul(
                        pc[:, qo:512],
                        v_sb[:, vbase:vbase + VW],
                        exps[t][:, s * 512: s * 512 + 512 - qo],
                        start=(t == 0), stop=(t == nsk - 1),
                        skip_group_check=True)

        for pi in range(nsk // 2):
            for t in (2 * pi, 2 * pi + 1):
                j = t - 4 * c
                qo = max(0, j) * 128
                w = 512 - qo
                pscore = sps.tile([128, 1024], F32)
                for s, hh in enumerate((h0, h1)):
                    nc.tensor.matmul(
                        pscore[:, s * 512: s * 512 + w],
                        kh[hh][:, t * 128:(t + 1) * 128],
                        qh[hh][:, c * 512 + qo:(c + 1) * 512],
                        start=True, stop=True,
                        skip_group_check=True)
                et = exp_pool.tile([128, 1024], BF16)
                if j <= 0:
                    nc.scalar.activation(et[:], pscore[:], EXP, scale=0.125)
                else:
                    for s in range(2):
                        nc.scalar.activation(
                            et[:, s * 512:s * 512 + w],
                            pscore[:, s * 512:s * 512 + w],
                            EXP, scale=0.125)
                if j >= 0:
                    # on ANY diagonal tile (key tile t == 4c+j) only the
                    # first 128 query columns can have key>query: beyond
                    # them f >= 128 > p-128j always satisfies causality.
                    # So the mask is a single [128,128] triangle per head.
                    for s in range(2):
                        nc.vector.tensor_mul(
                            et[:, s * 512:s * 512 + 128],
                            et[:, s * 512:s * 512 + 128],
                            mk_sb[:, 0:128])
                exps.append(et)
            if pi >= 2:
                pv_pair(2 * (pi - 2), 2 * pi - 3)
            for f in fillers.take():
                f()
        for pi in range(max(0, nsk // 2 - 2), nsk // 2):
            pv_pair(2 * pi, 2 * pi + 1)
        # fused normalize-drain: rowsums -> recip -> broadcast, then ONE mul
        # per head reads the PV psum and writes normalized bf16 ctx directly
        # (saves a separate [64,512] cast + in-place mul per head).  sums/rc1
        # ride the ACT stream (interleaves between this hp's and the next
        # hp's exps) except in chunk 3 hp0 where ACT is the binding engine.
        csl = slice(c * 512, (c + 1) * 512)
        on_act = not (c == NSC - 1 and hp == 0)
        for s, (hh, pc) in enumerate(((h0, pctx0), (h1, pctx1))):
            if on_act:
                nc.scalar.copy(sums_sb[hh * 32:hh * 32 + 1, csl],
                               pc[DK:DK + 1, :])
            else:
                nc.vector.tensor_copy(sums_sb[hh * 32:hh * 32 + 1, csl],
                                      pc[DK:DK + 1, :])
        # recip covers all 128 partitions (cost is free-dim only); the other
        # pair's rows are the 1.0 memset or recomputed identically later
        nc.vector.reciprocal_approx_fast(rcp_sb[:, csl], sums_sb[:, csl])
        for s, (hh, pc) in enumerate(((h0, pctx0), (h1, pctx1))):
            # cast+stage the recip row at partition 0: hw partition_broadcast
            # reads the tile's partition 0 regardless of the AP offset
            rc1 = nsb.tile([1, 512], BF16, tag="rc1")
            if on_act:
                nc.scalar.copy(rc1[:], rcp_sb[hh * 32:hh * 32 + 1, csl])
            else:
                nc.vector.tensor_copy(rc1[:], rcp_sb[hh * 32:hh * 32 + 1, csl])
            bc = nsb.tile([128, 512], BF16, tag="bc")
            nc.gpsimd.partition_broadcast(bc[:], rc1[:])
            nc.vector.tensor_mul(
                ctx_sb[s * 64:s * 64 + DK,
                       hp * S + c * 512: hp * S + (c + 1) * 512],
                pc[0:DK, :], bc[0:DK, :])

    class FillerFeed:
        """Spreads a chunk's filler units (next-chunk projections, deferred
        o_proj tiles) evenly across the attention pi-steps so the PE never
        front-loads them and then starves on the exp-bound tail."""

        def __init__(self, units, steps):
            self.units = list(units)
            self.steps = max(1, steps)
            self.step = 0
            self.given = 0

        def take(self):
            self.step += 1
            want = (len(self.units) * self.step) // self.steps
            out = self.units[self.given:want]
            self.given = want
            return out

        def rest(self):
            out = self.units[self.given:]
            self.given = len(self.units)
            return out

    # ---- fused emission: ascending chunks; projections for chunk c+1 and
    # deferred o_proj tiles ride along as PE filler inside chunk c's
    # exp-bound attention stream.  o_proj/v(3) are deferred so the ACT-
    # heaviest chunks (2, 3) still have PE filler available; chunk 3 gets
    # v(3) + o(1)[2:] + o(2), the tail only o(3).
    def interleave(a, b):
        out = []
        for i in range(max(len(a), len(b))):
            if i < len(a):
                out.append(a[i])
            if i < len(b):
                out.append(b[i])
        return out

    # qk units lead each feed: the next chunk's first scores gate on them,
    # so they must clear well before the chunk boundary.  v units precede
    # any PV that reads them (emission order is semantic order).
    feeds = {
        0: v_units(0) + qk_units(1) + v_units(1),
        1: qk_units(2) + interleave(v_units(2), o_units(range(0, 4))),
        2: qk_units(3) + o_units(range(4, 6)),
        3: v_units(3) + o_units(range(6, 12)),
    }
    for _u in qk_units(0):
        _u()
    late_dma[0]()  # xt chunk-1 second half
    late_dma[1]()  # sn remainder (before any qk(1) rope emission!)
    for c in range(NSC):
        nsk = 4 * (c + 1)
        # all fillers emitted by ~3/4 through the chunk so stragglers never
        # block the next chunk's scores at the boundary
        feed = FillerFeed(feeds[c], max(1, (3 * nsk) // 4))
        attention_hp(c, 0, feed)
        if c + 2 < len(late_dma):
            late_dma[c + 2]()  # xt chunk-(c+2) second half
        attention_hp(c, 1, feed)
        for f in feed.rest():
            f()
    for sm in range(12, 16):
        o_proj_sm(sm)
    _stack.close()


def build_nc():
    nc = bacc.Bacc("TRN2", target_bir_lowering=False, debug=False,
                   enable_asserts=False, num_devices=NCORES)
    xt = nc.dram_tensor("xt", [D, S], BF16, kind="ExternalInput").ap()
    wq = nc.dram_tensor("wq", [D, CW], BF16, kind="ExternalInput").ap()
    wk = nc.dram_tensor("wk", [D, CW], BF16, kind="ExternalInput").ap()
    wv = nc.dram_tensor("wv", [D, CW], BF16, kind="ExternalInput").ap()
    wo = nc.dram_tensor("wo", [CW, D], BF16, kind="ExternalInput").ap()
    cs = nc.dram_tensor("cs", [128, S], BF16, kind="ExternalInput").ap()
    sn = nc.dram_tensor("sn", [128, S], BF16, kind="ExternalInput").ap()
    mk = nc.dram_tensor("mk", [128, 1024], BF16, kind="ExternalInput").ap()
    pm = nc.dram_tensor("pm", [128, 128], BF16, kind="ExternalInput").ap()
    out = nc.dram_tensor("out_partial", [S, D], BF16, kind="ExternalOutput").ap()
    with tile.TileContext(nc) as tc:
        _build_kernel(tc, nc, xt, wq, wk, wv, wo, cs, sn, mk, pm, out)
    nc.compile()
    return nc


def _bf16(a):
    """Round-to-nearest-even f32 -> bfloat16 via bit tricks (fast, no ml_dtypes
    conversion loops)."""
    import ml_dtypes
    a = np.ascontiguousarray(a, dtype=np.float32)
    u = a.view(np.uint32)
    r = ((u >> 16) & 1) + np.uint32(0x7FFF)
    return ((u + r) >> 16).astype(np.uint16).view(ml_dtypes.bfloat16)


def make_in_maps(in_features, q_proj_weight, k_proj_weight, v_proj_weight,
                 o_proj_weight, token_positions):
    x = np.asarray(in_features, dtype=np.float32)
    wq = np.asarray(q_proj_weight, dtype=np.float32)
    wk = np.asarray(k_proj_weight, dtype=np.float32)
    wv = np.asarray(v_proj_weight, dtype=np.float32)
    wo = np.asarray(o_proj_weight, dtype=np.float32)
    pos = np.asarray(token_positions).astype(np.float64)

    inv = ROPE_THETA ** (-2.0 * np.arange(DK // 2, dtype=np.float64) / DK)
    ang = inv[:, None] * pos[None, :]  # [32, S]
    c32, s32 = np.cos(ang), np.sin(ang)
    # rows: per 64-row head block, [even(32); odd(32)]; repeats for 2 heads
    cs_full = _bf16(np.tile(c32, (4, 1)))
    sn_full = _bf16(np.concatenate([-s32, s32, -s32, s32], axis=0))

    p = np.arange(128)[:, None]
    f = np.arange(512)[None, :]
    # one triangle, duplicated for the two head streams of a psum pair
    mk = _bf16(np.tile((f >= p).astype(np.float32), (1, 2)))

    pm = _bf16(np.equal(np.arange(128)[:, None] ^ 32,
                        np.arange(128)[None, :]).astype(np.float32))

    xb = [np.ascontiguousarray(_bf16(x[b]).T) for b in range(B)]
    in_maps = []
    wq_c = {}
    for c in range(NCORES):
        b, g = c // 4, c % 4
        if g not in wq_c:
            cols = np.arange(g * CW, (g + 1) * CW)
            hcols = cols.reshape(HPC, DK)
            qcols = np.concatenate([np.concatenate([hcols[h, 0::2],
                                                    hcols[h, 1::2]])
                                    for h in range(HPC)])
            wq_c[g] = (
                np.ascontiguousarray(_bf16(wq[qcols, :]).T),
                np.ascontiguousarray(_bf16(wk[qcols, :]).T),
                np.ascontiguousarray(_bf16(wv[cols, :]).T),
                np.ascontiguousarray(_bf16(wo[:, cols]).T),
            )
        wq_g, wk_g, wv_g, wo_g = wq_c[g]
        in_maps.append({
            "xt": xb[b],
            "wq": wq_g,
            "wk": wk_g,
            "wv": wv_g,
            "wo": wo_g,
            "cs": cs_full,
            "sn": sn_full,
            "mk": mk,
            "pm": pm,
        })
    return in_maps


_NC_CACHE = []
last_exec_ns = None


def kernel(in_features, q_proj_weight, k_proj_weight, v_proj_weight,
           o_proj_weight, token_positions, d_model=1024, num_heads=16,
           **_ignored):
    global last_exec_ns
    assert int(d_model) == D and int(num_heads) == H
    in_maps = make_in_maps(in_features, q_proj_weight, k_proj_weight,
                           v_proj_weight, o_proj_weight, token_positions)
    if not _NC_CACHE:
        _NC_CACHE.append(build_nc())
    nc = _NC_CACHE[0]
    trace = bool(int(os.environ.get("KERNEL_TRACE", "0")))
    res = bass_utils.run_bass_kernel_spmd(nc, in_maps,
                                          core_ids=list(range(NCORES)),
                                          trace=trace)
    last_exec_ns = res.exec_time_ns
    parts = [np.asarray(r["out_partial"]).astype(np.float32)
             for r in res.results]
    out = np.stack([parts[0] + parts[1] + parts[2] + parts[3],
                    parts[4] + parts[5] + parts[6] + parts[7]])
    return out
